# revision 5
# baseline (speedup 1.0000x reference)
"""Category-specific linear on 8 trn2 cores — hidden-dim sharding, resident W.

v2 changes vs v1 (486us):
- bf16 output (halves out DMA + DVE bytes; adds ~2e-3 rel err, budget 2e-2).
- Per-m-tile DVE copies: each PSUM bank copied right after its 8-k
  accumulation, so the tail drain is one bank, not four.
- Out DMAs issued from the idle GpSimd queue -> no head-of-line blocking
  of x/W prefetch issues on the Sync queue.
- All W groups pre-issued early (1/sample + 3 seeded; SBUF holds all 16).
- First x pair + group-0 W arrive as interleaved k-chunks so the first
  matmul starts ~10us in; warmup shortened to 10 matmuls.
- x pairs prefetched 3 deep (bufs=4).
"""

import numpy as np
import ml_dtypes

B = 64
S = 512
DIN = 1024
DH = 4096
C = 16
NCORES = 8
NSH = DH // NCORES   # 512
P = 128
KO = DIN // P        # 8
MO = S // P          # 4

LAST_RESULTS = None


def _build_program(cats):
    import concourse.bacc as bacc
    import concourse.mybir as mybir
    import concourse.tile as tile

    nc = bacc.Bacc("TRN2", target_bir_lowering=False)

    assert B % 2 == 0
    x_d = nc.dram_tensor("x", (B // 2, P, 2, KO, S), mybir.dt.bfloat16,
                         kind="ExternalInput")
    w_d = nc.dram_tensor("w", (C, P, KO, NSH), mybir.dt.bfloat16,
                         kind="ExternalInput")
    out_d = nc.dram_tensor("out", (B, P, MO, NSH), mybir.dt.bfloat16,
                           kind="ExternalOutput")

    # Category-grouped processing order: samples of the same category are
    # consecutive, so each category's W slice is loaded once.
    used = []
    for c in cats:
        if c not in used:
            used.append(c)
    proc_order = [j for c in used for j in range(B) if cats[j] == c]
    n_groups = len(used)

    # k-chunking for the first x pair and group-0 W: the first matmuls
    # need only the first chunk, so compute starts as soon as ~768KB land.
    CHUNKS = [(0, 2), (2, 4), (4, 8)]

    with tile.TileContext(nc) as tc:
        with (
            tc.tile_pool(name="static", bufs=1) as wp,
            tc.tile_pool(name="wgroup", bufs=8) as wg,
            tc.tile_pool(name="xpool", bufs=5) as xb,
            tc.tile_pool(name="opool", bufs=3) as ob,
            tc.tile_pool(name="psum", bufs=2, space="PSUM") as ps,
        ):
            w_tiles = {}

            def emit_w(g):
                # W issues ride the idle Scalar queue: their buffer-reuse
                # waits must not block x prefetch issues on Sync.
                c = used[g]
                t = wg.tile([P, KO, NSH], mybir.dt.bfloat16, tag="w")
                nc.scalar.dma_start(t[:], w_d[c])
                w_tiles[c] = t

            # PE warmup: dummy matmuls with no DMA deps, run during the
            # ~9us DMA-subsystem init so the clock ramps before real data.
            warm_l = wp.tile([P, P], mybir.dt.bfloat16, tag="warm_l")
            warm_r = wp.tile([P, NSH], mybir.dt.bfloat16, tag="warm_r")
            nc.any.memzero(warm_l[:])
            nc.any.memzero(warm_r[:])
            warm_p = ps.tile([P, MO, NSH], mybir.dt.float32, tag="ps")
            for _ in range(10):
                nc.tensor.matmul(warm_p[:, 0, :], warm_l[:], warm_r[:],
                                 start=True, stop=True)

            # Head DMA sequence: interleaved k-chunks of (x pair 0, W c0).
            c0 = used[0]
            x0c = []   # chunk tiles for pair 0
            w0c = []   # chunk tiles for group-0 W
            for ci, (a, b) in enumerate(CHUNKS):
                tx = wp.tile([P, 2, b - a, S], mybir.dt.bfloat16,
                             tag=f"x0c{ci}")
                nc.sync.dma_start(tx[:], x_d[0, :, :, a:b, :])
                x0c.append(tx)
                tw = wp.tile([P, b - a, NSH], mybir.dt.bfloat16,
                             tag=f"w0c{ci}")
                nc.sync.dma_start(tw[:], w_d[c0, :, a:b, :])
                w0c.append(tw)

            def chunk_of(k):
                for ci, (a, b) in enumerate(CHUNKS):
                    if a <= k < b:
                        return ci, k - a
                raise AssertionError

            # x pairs 1..4 up front (pool bufs=5 -> 4-pair prefetch depth).
            xtiles = {}

            def emit_x(p):
                t = xb.tile([P, 2, KO, S], mybir.dt.bfloat16, tag="x")
                nc.sync.dma_start(t[:], x_d[p])
                xtiles[p] = t

            for p in (1, 2, 3, 4):
                if p < B // 2:
                    emit_x(p)

            # Seed W groups 1-4; the rest go out one per sample below.
            next_w = 1
            while next_w < min(5, n_groups):
                emit_w(next_w)
                next_w += 1

            for i in range(B):
                j = proc_order[i]
                c = cats[j]
                if next_w < n_groups:
                    emit_w(next_w)
                    next_w += 1
                if i % 2 == 0 and i >= 2:
                    p = i // 2 + 4
                    if p < B // 2:
                        emit_x(p)
                pt = ps.tile([P, MO, NSH], mybir.dt.float32, tag="ps")
                ot = ob.tile([P, MO, NSH], mybir.dt.bfloat16, tag="o")
                for m in range(MO):
                    for k in range(KO):
                        if i < 2:
                            ci, kk = chunk_of(k)
                            lhs = x0c[ci][:, i, kk, m * P:(m + 1) * P]
                        else:
                            lhs = xtiles[i // 2][:, i % 2, k, m * P:(m + 1) * P]
                        if c == c0:
                            ci, kk = chunk_of(k)
                            rhs = w0c[ci][:, kk, :]
                        else:
                            rhs = w_tiles[c][:, k, :]
                        nc.tensor.matmul(
                            pt[:, m, :],
                            lhs,
                            rhs,
                            start=(k == 0),
                            stop=(k == KO - 1),
                        )
                    nc.vector.tensor_copy(ot[:, m, :], pt[:, m, :])
                nc.gpsimd.dma_start(out_d[i], ot[:])

    nc.compile()
    return nc


def kernel(x, cat_ids, W, b):
    global LAST_RESULTS
    from concourse import bass_utils

    x = np.asarray(x, dtype=np.float32)
    cat_ids_np = np.asarray(cat_ids).astype(np.int64)
    W = np.asarray(W, dtype=np.float32)
    b = np.asarray(b, dtype=np.float32)
    cats = [int(c) for c in cat_ids_np]

    # Same category-grouped order the program bakes in.
    used = []
    for c in cats:
        if c not in used:
            used.append(c)
    proc_order = [j for c in used for j in range(B) if cats[j] == c]

    # x: [B,S,DIN] -> [B,P(q),KO,S] bf16, in processed order, paired.
    xp = np.ascontiguousarray(
        x.reshape(B, S, KO, P).transpose(0, 3, 2, 1)
    ).astype(ml_dtypes.bfloat16)[proc_order]
    xp = np.ascontiguousarray(
        xp.reshape(B // 2, 2, P, KO, S).transpose(0, 2, 1, 3, 4)
    )

    in_maps = []
    for core in range(NCORES):
        Wc = W[:, :, core * NSH:(core + 1) * NSH]
        Wp = np.ascontiguousarray(
            Wc.reshape(C, KO, P, NSH).transpose(0, 2, 1, 3)
        ).astype(ml_dtypes.bfloat16)
        in_maps.append({"x": xp, "w": Wp})

    nc = _build_program(cats)
    res = bass_utils.run_bass_kernel_spmd(
        nc, in_maps, core_ids=list(range(NCORES))
    )
    LAST_RESULTS = res

    inv = np.argsort(np.asarray(proc_order))
    out = np.empty((B, S, DH), dtype=np.float32)
    for core in range(NCORES):
        oc = np.asarray(res.results[core]["out"]).astype(np.float32)
        # out[proc_order[i], m*128+p, n] = oc[i, p, m, n]
        oc = oc.reshape(B, P, MO, NSH).transpose(0, 2, 1, 3).reshape(B, S, NSH)
        out[:, :, core * NSH:(core + 1) * NSH] = oc[inv]

    if b.any():
        out += b[cats][:, None, :]
    return out


# revision 6
# speedup vs baseline: 1.0903x; 1.0903x over previous
"""Category-specific linear on 8 trn2 cores — hidden-dim sharding, resident W.

v3 (from v1 @488us, v2 @518us):
- bf16 output (halves out DMA + DVE bytes; total rel err ~3.7e-3 vs 2e-2).
- Out DMAs issued from the idle GpSimd queue -> their copy-done waits no
  longer block x/W prefetch issues on the Sync queue.
- Groups processed LARGEST category first: the first W tile covers the
  most compute, so the slow first ~25us of DMA (subsystem ramp) carries
  almost only x; small groups run late when the pipe is warm.
- All x/W DMAs on one Sync queue in exact need order; W tiles rotate
  through an 8-buffer pool (issues self-pace via buffer-reuse waits).
- First x pair + first W arrive as interleaved k-chunks so compute can
  start as soon as ~0.8MiB lands.
- Last sample: per-m-tile copies + DMAs to shrink the tail drain.
"""

import numpy as np
import ml_dtypes

B = 64
S = 512
DIN = 1024
DH = 4096
C = 16
NCORES = 8
NSH = DH // NCORES   # 512
P = 128
KO = DIN // P        # 8
MO = S // P          # 4

LAST_RESULTS = None


def _plan(cats):
    """Largest-first category order; proc_order[i] = original sample idx."""
    first = {}
    for j, c in enumerate(cats):
        first.setdefault(c, j)
    counts = {c: cats.count(c) for c in first}
    used = sorted(first, key=lambda c: (-counts[c], first[c]))
    proc_order = [j for c in used for j in range(len(cats)) if cats[j] == c]
    group_start = {}
    for i, j in enumerate(proc_order):
        group_start.setdefault(cats[j], i)
    return used, proc_order, group_start


def _build_program(cats):
    import concourse.bacc as bacc
    import concourse.mybir as mybir
    import concourse.tile as tile

    nc = bacc.Bacc("TRN2", target_bir_lowering=False)

    assert B % 2 == 0
    x_d = nc.dram_tensor("x", (B // 2, P, 2, KO, S), mybir.dt.bfloat16,
                         kind="ExternalInput")
    w_d = nc.dram_tensor("w", (C, P, KO, NSH), mybir.dt.bfloat16,
                         kind="ExternalInput")
    out_d = nc.dram_tensor("out", (B, P, MO, NSH), mybir.dt.bfloat16,
                           kind="ExternalOutput")

    used, proc_order, group_start = _plan(cats)
    n_groups = len(used)

    # k-chunking for the first x pair and group-0 W: compute starts as
    # soon as the first ~0.8MiB lands (DMA subsystem ramps slowly).
    CHUNKS = [(0, 2), (2, 4), (4, 8)]

    with tile.TileContext(nc) as tc:
        with (
            tc.tile_pool(name="static", bufs=1) as wp,
            tc.tile_pool(name="wgroup", bufs=8) as wg,
            tc.tile_pool(name="xpool", bufs=5) as xb,
            tc.tile_pool(name="opool", bufs=3) as ob,
            tc.tile_pool(name="psum", bufs=2, space="PSUM") as ps,
        ):
            w_tiles = {}

            def emit_w(g):
                c = used[g]
                t = wg.tile([P, KO, NSH], mybir.dt.bfloat16, tag="w")
                nc.sync.dma_start(t[:], w_d[c])
                w_tiles[c] = t

            # PE warmup: dummy matmuls with no DMA deps, run during the
            # ~10us DMA-subsystem init so the clock ramps before real data.
            warm_l = wp.tile([P, P], mybir.dt.bfloat16, tag="warm_l")
            warm_r = wp.tile([P, NSH], mybir.dt.bfloat16, tag="warm_r")
            nc.any.memzero(warm_l[:])
            nc.any.memzero(warm_r[:])
            warm_p = ps.tile([P, MO, NSH], mybir.dt.float32, tag="ps")
            for _ in range(10):
                nc.tensor.matmul(warm_p[:, 0, :], warm_l[:], warm_r[:],
                                 start=True, stop=True)

            # Head DMA sequence: interleaved k-chunks of (x pair 0, W c0),
            # in exact need order on one queue.
            c0 = used[0]
            x0c = []
            w0c = []
            for ci, (a, b) in enumerate(CHUNKS):
                tx = wp.tile([P, 2, b - a, S], mybir.dt.bfloat16,
                             tag=f"x0c{ci}")
                nc.sync.dma_start(tx[:], x_d[0, :, :, a:b, :])
                x0c.append(tx)
                tw = wp.tile([P, b - a, NSH], mybir.dt.bfloat16,
                             tag=f"w0c{ci}")
                nc.sync.dma_start(tw[:], w_d[c0, :, a:b, :])
                w0c.append(tw)

            def chunk_of(k):
                for ci, (a, b) in enumerate(CHUNKS):
                    if a <= k < b:
                        return ci, k - a
                raise AssertionError

            xtiles = {}

            def emit_x(p):
                t = xb.tile([P, 2, KO, S], mybir.dt.bfloat16, tag="x")
                nc.sync.dma_start(t[:], x_d[p])
                xtiles[p] = t

            # Pairs 1-2 right behind the head chunks; rest deadline-paced.
            for p in (1, 2):
                if p < B // 2:
                    emit_x(p)

            # W group issue sample: 4 samples before first use.
            w_sched = {}
            for g in range(1, n_groups):
                w_sched.setdefault(max(0, group_start[used[g]] - 4), []).append(g)

            for i in range(B):
                j = proc_order[i]
                c = cats[j]
                for g in w_sched.get(i, ()):
                    emit_w(g)
                if i % 2 == 0:
                    p = i // 2 + 3
                    if p < B // 2:
                        emit_x(p)
                pt = ps.tile([P, MO, NSH], mybir.dt.float32, tag="ps")
                ot = ob.tile([P, MO, NSH], mybir.dt.bfloat16, tag="o")
                last = i == B - 1
                for m in range(MO):
                    for k in range(KO):
                        if i < 2:
                            ci, kk = chunk_of(k)
                            lhs = x0c[ci][:, i, kk, m * P:(m + 1) * P]
                        else:
                            lhs = xtiles[i // 2][:, i % 2, k, m * P:(m + 1) * P]
                        if c == c0:
                            ci, kk = chunk_of(k)
                            rhs = w0c[ci][:, kk, :]
                        else:
                            rhs = w_tiles[c][:, k, :]
                        nc.tensor.matmul(
                            pt[:, m, :],
                            lhs,
                            rhs,
                            start=(k == 0),
                            stop=(k == KO - 1),
                        )
                    if last:
                        # tail: drain each PSUM bank as soon as it's done
                        nc.vector.tensor_copy(ot[:, m, :], pt[:, m, :])
                        nc.gpsimd.dma_start(out_d[i, :, m, :], ot[:, m, :])
                if not last:
                    nc.vector.tensor_copy(ot[:], pt[:])
                    nc.gpsimd.dma_start(out_d[i], ot[:])

    nc.compile()
    return nc


def kernel(x, cat_ids, W, b):
    global LAST_RESULTS
    from concourse import bass_utils

    x = np.asarray(x, dtype=np.float32)
    cat_ids_np = np.asarray(cat_ids).astype(np.int64)
    W = np.asarray(W, dtype=np.float32)
    b = np.asarray(b, dtype=np.float32)
    cats = [int(c) for c in cat_ids_np]

    used, proc_order, _ = _plan(cats)

    # x: [B,S,DIN] -> [B,P(q),KO,S] bf16, in processed order, paired.
    xp = np.ascontiguousarray(
        x.reshape(B, S, KO, P).transpose(0, 3, 2, 1)
    ).astype(ml_dtypes.bfloat16)[proc_order]
    xp = np.ascontiguousarray(
        xp.reshape(B // 2, 2, P, KO, S).transpose(0, 2, 1, 3, 4)
    )

    in_maps = []
    for core in range(NCORES):
        Wc = W[:, :, core * NSH:(core + 1) * NSH]
        Wp = np.ascontiguousarray(
            Wc.reshape(C, KO, P, NSH).transpose(0, 2, 1, 3)
        ).astype(ml_dtypes.bfloat16)
        in_maps.append({"x": xp, "w": Wp})

    nc = _build_program(cats)
    res = bass_utils.run_bass_kernel_spmd(
        nc, in_maps, core_ids=list(range(NCORES))
    )
    LAST_RESULTS = res

    inv = np.argsort(np.asarray(proc_order))
    out = np.empty((B, S, DH), dtype=np.float32)
    for core in range(NCORES):
        oc = np.asarray(res.results[core]["out"]).astype(np.float32)
        # out[proc_order[i], m*128+p, n] = oc[i, p, m, n]
        oc = oc.reshape(B, P, MO, NSH).transpose(0, 2, 1, 3).reshape(B, S, NSH)
        out[:, :, core * NSH:(core + 1) * NSH] = oc[inv]

    if b.any():
        out += b[cats][:, None, :]
    return out


# revision 8
# speedup vs baseline: 1.0969x; 1.0061x over previous
"""Category-specific linear on 8 trn2 cores — hidden-dim sharding, resident W.

v3 (from v1 @488us, v2 @518us):
- bf16 output (halves out DMA + DVE bytes; total rel err ~3.7e-3 vs 2e-2).
- Out DMAs issued from the idle GpSimd queue -> their copy-done waits no
  longer block x/W prefetch issues on the Sync queue.
- Groups processed LARGEST category first: the first W tile covers the
  most compute, so the slow first ~25us of DMA (subsystem ramp) carries
  almost only x; small groups run late when the pipe is warm.
- All x/W DMAs on one Sync queue in exact need order; W tiles rotate
  through an 8-buffer pool (issues self-pace via buffer-reuse waits).
- First x pair + first W arrive as interleaved k-chunks so compute can
  start as soon as ~0.8MiB lands.
- Last sample: per-m-tile copies + DMAs to shrink the tail drain.
"""

import numpy as np
import ml_dtypes

B = 64
S = 512
DIN = 1024
DH = 4096
C = 16
NCORES = 8
NSH = DH // NCORES   # 512
P = 128
KO = DIN // P        # 8
MO = S // P          # 4

LAST_RESULTS = None


def _plan(cats):
    """Largest-first category order; proc_order[i] = original sample idx."""
    first = {}
    for j, c in enumerate(cats):
        first.setdefault(c, j)
    counts = {c: cats.count(c) for c in first}
    used = sorted(first, key=lambda c: (-counts[c], first[c]))
    proc_order = [j for c in used for j in range(len(cats)) if cats[j] == c]
    group_start = {}
    for i, j in enumerate(proc_order):
        group_start.setdefault(cats[j], i)
    return used, proc_order, group_start


def _build_program(cats):
    import concourse.bacc as bacc
    import concourse.mybir as mybir
    import concourse.tile as tile

    nc = bacc.Bacc("TRN2", target_bir_lowering=False)

    assert B % 2 == 0
    x_d = nc.dram_tensor("x", (B // 2, P, 2, KO, S), mybir.dt.bfloat16,
                         kind="ExternalInput")
    w_d = nc.dram_tensor("w", (C, P, KO, NSH), mybir.dt.bfloat16,
                         kind="ExternalInput")
    out_d = nc.dram_tensor("out", (B, P, MO, NSH), mybir.dt.bfloat16,
                           kind="ExternalOutput")

    used, proc_order, group_start = _plan(cats)
    n_groups = len(used)

    # k-chunking for the first x pair and group-0 W: compute starts as
    # soon as the first ~0.4MiB lands (DMA subsystem ramps slowly).
    CHUNKS = [(0, 1), (1, 2), (2, 4), (4, 8)]

    with tile.TileContext(nc) as tc:
        with (
            tc.tile_pool(name="static", bufs=1) as wp,
            tc.tile_pool(name="wgroup", bufs=8) as wg,
            tc.tile_pool(name="xpool", bufs=5) as xb,
            tc.tile_pool(name="opool", bufs=3) as ob,
            tc.tile_pool(name="psum", bufs=2, space="PSUM") as ps,
        ):
            w_tiles = {}

            def emit_w(g):
                c = used[g]
                t = wg.tile([P, KO, NSH], mybir.dt.bfloat16, tag="w")
                nc.sync.dma_start(t[:], w_d[c])
                w_tiles[c] = t

            # PE warmup: dummy matmuls with no DMA deps, run during the
            # ~10us DMA-subsystem init so the clock ramps before real data.
            warm_l = wp.tile([P, P], mybir.dt.bfloat16, tag="warm_l")
            warm_r = wp.tile([P, NSH], mybir.dt.bfloat16, tag="warm_r")
            nc.any.memzero(warm_l[:])
            nc.any.memzero(warm_r[:])
            warm_p = ps.tile([P, MO, NSH], mybir.dt.float32, tag="ps")
            for _ in range(10):
                nc.tensor.matmul(warm_p[:, 0, :], warm_l[:], warm_r[:],
                                 start=True, stop=True)

            # Head DMA sequence: interleaved k-chunks of (x pair 0, W c0),
            # in exact need order on one queue.
            c0 = used[0]
            x0c = []
            w0c = []
            for ci, (a, b) in enumerate(CHUNKS):
                tx = wp.tile([P, 2, b - a, S], mybir.dt.bfloat16,
                             tag=f"x0c{ci}")
                nc.sync.dma_start(tx[:], x_d[0, :, :, a:b, :])
                x0c.append(tx)
                tw = wp.tile([P, b - a, NSH], mybir.dt.bfloat16,
                             tag=f"w0c{ci}")
                nc.sync.dma_start(tw[:], w_d[c0, :, a:b, :])
                w0c.append(tw)

            def chunk_of(k):
                for ci, (a, b) in enumerate(CHUNKS):
                    if a <= k < b:
                        return ci, k - a
                raise AssertionError

            xtiles = {}

            def emit_x(p):
                t = xb.tile([P, 2, KO, S], mybir.dt.bfloat16, tag="x")
                nc.sync.dma_start(t[:], x_d[p])
                xtiles[p] = t

            # Pairs 1-2 right behind the head chunks; rest deadline-paced.
            for p in (1, 2):
                if p < B // 2:
                    emit_x(p)

            # W group issue sample: 4 samples before first use.
            w_sched = {}
            for g in range(1, n_groups):
                w_sched.setdefault(max(0, group_start[used[g]] - 4), []).append(g)

            for i in range(B):
                j = proc_order[i]
                c = cats[j]
                for g in w_sched.get(i, ()):
                    emit_w(g)
                if i % 2 == 0:
                    p = i // 2 + 3
                    if p < B // 2:
                        emit_x(p)
                pt = ps.tile([P, MO, NSH], mybir.dt.float32, tag="ps")
                ot = ob.tile([P, MO, NSH], mybir.dt.bfloat16, tag="o")
                for m in range(MO):
                    for k in range(KO):
                        if i < 2:
                            ci, kk = chunk_of(k)
                            lhs = x0c[ci][:, i, kk, m * P:(m + 1) * P]
                        else:
                            lhs = xtiles[i // 2][:, i % 2, k, m * P:(m + 1) * P]
                        if c == c0:
                            ci, kk = chunk_of(k)
                            rhs = w0c[ci][:, kk, :]
                        else:
                            rhs = w_tiles[c][:, k, :]
                        nc.tensor.matmul(
                            pt[:, m, :],
                            lhs,
                            rhs,
                            start=(k == 0),
                            stop=(k == KO - 1),
                        )
                # Copies ride the otherwise-idle Scalar (ACT) engine.
                nc.scalar.copy(ot[:], pt[:])
                if i == B - 1:
                    # Sync's DMA queue is several times wider than GpSimd's
                    # and idle by now — shortest possible tail drain.
                    nc.sync.dma_start(out_d[i], ot[:])
                else:
                    nc.gpsimd.dma_start(out_d[i], ot[:])

    nc.compile()
    return nc


def kernel(x, cat_ids, W, b):
    global LAST_RESULTS
    from concourse import bass_utils

    x = np.asarray(x, dtype=np.float32)
    cat_ids_np = np.asarray(cat_ids).astype(np.int64)
    W = np.asarray(W, dtype=np.float32)
    b = np.asarray(b, dtype=np.float32)
    cats = [int(c) for c in cat_ids_np]

    used, proc_order, _ = _plan(cats)

    # x: [B,S,DIN] -> [B,P(q),KO,S] bf16, in processed order, paired.
    xp = np.ascontiguousarray(
        x.reshape(B, S, KO, P).transpose(0, 3, 2, 1)
    ).astype(ml_dtypes.bfloat16)[proc_order]
    xp = np.ascontiguousarray(
        xp.reshape(B // 2, 2, P, KO, S).transpose(0, 2, 1, 3, 4)
    )

    in_maps = []
    for core in range(NCORES):
        Wc = W[:, :, core * NSH:(core + 1) * NSH]
        Wp = np.ascontiguousarray(
            Wc.reshape(C, KO, P, NSH).transpose(0, 2, 1, 3)
        ).astype(ml_dtypes.bfloat16)
        in_maps.append({"x": xp, "w": Wp})

    nc = _build_program(cats)
    res = bass_utils.run_bass_kernel_spmd(
        nc, in_maps, core_ids=list(range(NCORES))
    )
    LAST_RESULTS = res

    inv = np.argsort(np.asarray(proc_order))
    out = np.empty((B, S, DH), dtype=np.float32)
    for core in range(NCORES):
        oc = np.asarray(res.results[core]["out"]).astype(np.float32)
        # out[proc_order[i], m*128+p, n] = oc[i, p, m, n]
        oc = oc.reshape(B, P, MO, NSH).transpose(0, 2, 1, 3).reshape(B, S, NSH)
        out[:, :, core * NSH:(core + 1) * NSH] = oc[inv]

    if b.any():
        out += b[cats][:, None, :]
    return out


# revision 11
# speedup vs baseline: 1.1022x; 1.0048x over previous
"""Category-specific linear on 8 trn2 cores — hidden-dim sharding, resident W.

v3 (from v1 @488us, v2 @518us):
- bf16 output (halves out DMA + DVE bytes; total rel err ~3.7e-3 vs 2e-2).
- Out DMAs issued from the idle GpSimd queue -> their copy-done waits no
  longer block x/W prefetch issues on the Sync queue.
- Groups processed LARGEST category first: the first W tile covers the
  most compute, so the slow first ~25us of DMA (subsystem ramp) carries
  almost only x; small groups run late when the pipe is warm.
- All x/W DMAs on one Sync queue in exact need order; W tiles rotate
  through an 8-buffer pool (issues self-pace via buffer-reuse waits).
- First x pair + first W arrive as interleaved k-chunks so compute can
  start as soon as ~0.8MiB lands.
- Last sample: per-m-tile copies + DMAs to shrink the tail drain.
"""

import numpy as np
import ml_dtypes

B = 64
S = 512
DIN = 1024
DH = 4096
C = 16
NCORES = 8
NSH = DH // NCORES   # 512
P = 128
KO = DIN // P        # 8
MO = S // P          # 4

LAST_RESULTS = None


def _plan(cats):
    """Largest-first category order; proc_order[i] = original sample idx."""
    first = {}
    for j, c in enumerate(cats):
        first.setdefault(c, j)
    counts = {c: cats.count(c) for c in first}
    used = sorted(first, key=lambda c: (-counts[c], first[c]))
    proc_order = [j for c in used for j in range(len(cats)) if cats[j] == c]
    group_start = {}
    for i, j in enumerate(proc_order):
        group_start.setdefault(cats[j], i)
    return used, proc_order, group_start


def _build_program(cats):
    import concourse.bacc as bacc
    import concourse.mybir as mybir
    import concourse.tile as tile

    nc = bacc.Bacc("TRN2", target_bir_lowering=False)

    assert B % 2 == 0
    x_d = nc.dram_tensor("x", (B // 2, P, 2, KO, S), mybir.dt.bfloat16,
                         kind="ExternalInput")
    w_d = nc.dram_tensor("w", (C, P, KO, NSH), mybir.dt.bfloat16,
                         kind="ExternalInput")
    out_d = nc.dram_tensor("out", (B, P, MO, NSH), mybir.dt.bfloat16,
                           kind="ExternalOutput")

    used, proc_order, group_start = _plan(cats)
    n_groups = len(used)

    # k-chunking for the first x pair and group-0 W: compute starts as
    # soon as the first ~0.4MiB lands (DMA subsystem ramps slowly).
    CHUNKS = [(0, 1), (1, 2), (2, 4), (4, 6), (6, 8)]

    with tile.TileContext(nc) as tc:
        with (
            tc.tile_pool(name="static", bufs=1) as wp,
            tc.tile_pool(name="wgroup", bufs=8) as wg,
            tc.tile_pool(name="xpool", bufs=5) as xb,
            tc.tile_pool(name="opool", bufs=3) as ob,
            tc.tile_pool(name="psum", bufs=2, space="PSUM") as ps,
        ):
            w_tiles = {}

            def emit_w(g):
                c = used[g]
                t = wg.tile([P, KO, NSH], mybir.dt.bfloat16, tag="w")
                nc.sync.dma_start(t[:], w_d[c])
                w_tiles[c] = t

            # PE warmup: dummy matmuls with no DMA deps, run during the
            # ~10us DMA-subsystem init so the clock ramps before real data.
            warm_l = wp.tile([P, P], mybir.dt.bfloat16, tag="warm_l")
            warm_r = wp.tile([P, NSH], mybir.dt.bfloat16, tag="warm_r")
            nc.any.memzero(warm_l[:])
            nc.any.memzero(warm_r[:])
            warm_p = ps.tile([P, MO, NSH], mybir.dt.float32, tag="ps")
            for _ in range(5):
                nc.tensor.matmul(warm_p[:, 0, :], warm_l[:], warm_r[:],
                                 start=True, stop=True)

            # Head DMA sequence: interleaved k-chunks of (x pair 0, W c0),
            # in exact need order on one queue.
            c0 = used[0]
            x0c = []
            w0c = []
            for ci, (a, b) in enumerate(CHUNKS):
                tx = wp.tile([P, 2, b - a, S], mybir.dt.bfloat16,
                             tag=f"x0c{ci}")
                nc.sync.dma_start(tx[:], x_d[0, :, :, a:b, :])
                x0c.append(tx)
                tw = wp.tile([P, b - a, NSH], mybir.dt.bfloat16,
                             tag=f"w0c{ci}")
                nc.sync.dma_start(tw[:], w_d[c0, :, a:b, :])
                w0c.append(tw)

            def chunk_of(k):
                for ci, (a, b) in enumerate(CHUNKS):
                    if a <= k < b:
                        return ci, k - a
                raise AssertionError

            xtiles = {}

            def emit_x(p):
                t = xb.tile([P, 2, KO, S], mybir.dt.bfloat16, tag="x")
                nc.sync.dma_start(t[:], x_d[p])
                xtiles[p] = t

            # Pairs 1-2 right behind the head chunks; rest deadline-paced.
            for p in (1, 2):
                if p < B // 2:
                    emit_x(p)

            # W group issue sample: 4 samples before first use.
            w_sched = {}
            for g in range(1, n_groups):
                w_sched.setdefault(max(0, group_start[used[g]] - 4), []).append(g)

            for i in range(B):
                j = proc_order[i]
                c = cats[j]
                for g in w_sched.get(i, ()):
                    emit_w(g)
                if i % 2 == 0:
                    p = i // 2 + 3
                    if p < B // 2:
                        emit_x(p)
                pt = ps.tile([P, MO, NSH], mybir.dt.float32, tag="ps")
                ot = ob.tile([P, MO, NSH], mybir.dt.bfloat16, tag="o")
                for m in range(MO):
                    for k in range(KO):
                        if i < 2:
                            ci, kk = chunk_of(k)
                            lhs = x0c[ci][:, i, kk, m * P:(m + 1) * P]
                        else:
                            lhs = xtiles[i // 2][:, i % 2, k, m * P:(m + 1) * P]
                        if c == c0:
                            ci, kk = chunk_of(k)
                            rhs = w0c[ci][:, kk, :]
                        else:
                            rhs = w_tiles[c][:, k, :]
                        nc.tensor.matmul(
                            pt[:, m, :],
                            lhs,
                            rhs,
                            start=(k == 0),
                            stop=(k == KO - 1),
                        )
                if i == B - 1:
                    # Tail drain: split the copy across both idle engines
                    # and the DMA across two transfers on Sync's wide queue.
                    nc.scalar.copy(ot[:, 0:2, :], pt[:, 0:2, :])
                    nc.vector.tensor_copy(ot[:, 2:4, :], pt[:, 2:4, :])
                    nc.sync.dma_start(out_d[i, :, 0:2, :], ot[:, 0:2, :])
                    nc.sync.dma_start(out_d[i, :, 2:4, :], ot[:, 2:4, :])
                else:
                    # Copies ride the otherwise-idle Scalar (ACT) engine.
                    nc.scalar.copy(ot[:], pt[:])
                    nc.gpsimd.dma_start(out_d[i], ot[:])

    nc.compile()
    return nc


def kernel(x, cat_ids, W, b):
    global LAST_RESULTS
    from concourse import bass_utils

    x = np.asarray(x, dtype=np.float32)
    cat_ids_np = np.asarray(cat_ids).astype(np.int64)
    W = np.asarray(W, dtype=np.float32)
    b = np.asarray(b, dtype=np.float32)
    cats = [int(c) for c in cat_ids_np]

    used, proc_order, _ = _plan(cats)

    # x: [B,S,DIN] -> [B,P(q),KO,S] bf16, in processed order, paired.
    xp = np.ascontiguousarray(
        x.reshape(B, S, KO, P).transpose(0, 3, 2, 1)
    ).astype(ml_dtypes.bfloat16)[proc_order]
    xp = np.ascontiguousarray(
        xp.reshape(B // 2, 2, P, KO, S).transpose(0, 2, 1, 3, 4)
    )

    in_maps = []
    for core in range(NCORES):
        Wc = W[:, :, core * NSH:(core + 1) * NSH]
        Wp = np.ascontiguousarray(
            Wc.reshape(C, KO, P, NSH).transpose(0, 2, 1, 3)
        ).astype(ml_dtypes.bfloat16)
        in_maps.append({"x": xp, "w": Wp})

    nc = _build_program(cats)
    res = bass_utils.run_bass_kernel_spmd(
        nc, in_maps, core_ids=list(range(NCORES))
    )
    LAST_RESULTS = res

    inv = np.argsort(np.asarray(proc_order))
    out = np.empty((B, S, DH), dtype=np.float32)
    for core in range(NCORES):
        oc = np.asarray(res.results[core]["out"]).astype(np.float32)
        # out[proc_order[i], m*128+p, n] = oc[i, p, m, n]
        oc = oc.reshape(B, P, MO, NSH).transpose(0, 2, 1, 3).reshape(B, S, NSH)
        out[:, :, core * NSH:(core + 1) * NSH] = oc[inv]

    if b.any():
        out += b[cats][:, None, :]
    return out
